# revision 2
# baseline (speedup 1.0000x reference)
"""Trainium2 Bass kernel for nn_EqvarLayer (GNN message passing).

Computes, for the reference module:
    px1    = px @ W_pp.T                      (node-level linear)
    ix     = (px1[idx_i] @ W_i.T + px1[idx_j] @ W_j.T) * diff @ W_ii.T
    px_out = segment_sum(ix, idx_i)

All linear maps commute with the gather and the per-(edge,v) diff scale, so
with C_i = W_ii@W_i@W_pp and C_j = W_ii@W_j@W_pp (folded on host):
    ix[e,v,:]  = diff[e,v] * (pxA[idx_i[e],v,:] + pxB[idx_j[e],v,:])
    pxA = px @ C_i.T ; pxB = px @ C_j.T       (node-level, computed on device)

Edge-parallel across 8 NeuronCores: edges are sorted by idx_i on the host and
sharded contiguously.  Within a core the sorted edges are grouped into node
"blocks" (<=128-node span, fixed tile count T).  Per 128-edge tile:
  - idx_j side: one [128,1]-indexed indirect DMA gathers pxB rows (768B each)
  - idx_i side: a one-hot matmul against an SBUF-resident pxA panel (exact:
    pxA is kept as a bf16 hi+lo pair; two accumulating matmuls reconstruct
    the f32 value in PSUM)
  - stag = (pxA_g + pxB_g) * diff  on DVE (broadcast AP for the diff expand)
  - segment-sum: one-hot^T matmul accumulated in PSUM across the block's
    tiles, flushed once per block (no DRAM read-modify-write at all)
The scatter output is returned block-major and assembled on the host.
"""

import numpy as np
import ml_dtypes

N_NODES = 50000
N_EDGES = 800000
F_DIM = 64
FH = 3 * F_DIM            # 192 floats per node/edge row
NC = 8                    # cores
P = 128
T_TILES = 14              # tiles (of 128 edges) per block
CAP = T_TILES * P         # max edges per block
P1_TILE = 512             # phase-1 node tile (columns per matmul)

BF16 = ml_dtypes.bfloat16


def _split_hilo(x32):
    hi = x32.astype(BF16)
    lo = (x32 - hi.astype(np.float32)).astype(BF16)
    return hi, lo


def _build_blocks(si):
    """Partition sorted node ids si into blocks of <=CAP edges spanning
    <128 node ids.  Returns list of (n0, start, end)."""
    blocks = []
    s = 0
    n = len(si)
    while s < n:
        n0 = int(si[s])
        hi = int(np.searchsorted(si, n0 + P, side="left"))
        e = min(s + CAP, hi)
        blocks.append((n0, s, e))
        s = e
    return blocks


def _wrap_tiles(arr, inner=None):
    """[L(,inner)] -> [128, L//128(,inner)], edge k*128+p at [p,k]."""
    if inner is None:
        return np.ascontiguousarray(arr.reshape(-1, P).T)
    return np.ascontiguousarray(arr.reshape(-1, P, inner).transpose(1, 0, 2))


def _stack_pxT(pxf, cols_idx=None, ncols=None):
    """Build the [128, 3, ncols] bf16 stacked (hi/lo interleaved) feature-major
    layout from px rows.  pxf: [N, 192] f32.  Row r of v-block: r<64 -> hi of
    feature r, r>=64 -> lo of feature r-64."""
    if cols_idx is None:
        sel = pxf
    else:
        sel = np.zeros((len(cols_idx), FH), np.float32)
        valid = (cols_idx >= 0) & (cols_idx < pxf.shape[0])
        sel[valid] = pxf[cols_idx[valid]]
    n = sel.shape[0]
    if ncols is None:
        ncols = n
    xT = sel.T.reshape(3, F_DIM, n)                  # [v, f, n]
    hi = xT.astype(BF16)
    lo = (xT - hi.astype(np.float32)).astype(BF16)
    out = np.zeros((P, 3, ncols), BF16)
    out[:F_DIM, :, :n] = hi.transpose(1, 0, 2)       # [f, v, n]
    out[F_DIM:, :, :n] = lo.transpose(1, 0, 2)
    return out


def _weight_consts(C):
    """lhsT constants for the hi/lo matmul pair of out = C @ x."""
    Chi = C.astype(BF16)
    Clo = (C - Chi.astype(np.float32)).astype(BF16)
    stk = np.zeros((P, F_DIM), BF16)
    stk[:F_DIM] = Chi.T                               # [f, g]
    stk[F_DIM:] = Chi.T
    return np.ascontiguousarray(stk), np.ascontiguousarray(Clo.T)


def _build_bass(B, NT1B, NPXB):
    """Build the SPMD bass program.  B: blocks per core; NT1B: phase-1b node
    tiles; NPXB: padded pxB row count."""
    import concourse.bass as bass
    import concourse.bacc as bacc
    import concourse.mybir as mybir
    import concourse.tile as tile
    from concourse.masks import make_identity

    dt = mybir.dt
    PB = (B + 3) // 4                 # phase-1a 512-node tiles
    PAN = PB * 4                      # panel block slots (padded to x4)

    nc = bacc.Bacc(None, target_bir_lowering=False,
                   dynamic_dma_scratch_size=32768)

    # ---- inputs ----
    d_pxT = nc.dram_tensor("pxT_stk", [P, 3, NT1B * P1_TILE], dt.bfloat16,
                           kind="ExternalInput")
    d_pxTa = nc.dram_tensor("pxTa_stk", [P, 3, PAN * P], dt.bfloat16,
                            kind="ExternalInput")
    d_cis = nc.dram_tensor("ci_stk", [P, F_DIM], dt.bfloat16, kind="ExternalInput")
    d_cil = nc.dram_tensor("ci_lo", [F_DIM, F_DIM], dt.bfloat16, kind="ExternalInput")
    d_cjs = nc.dram_tensor("cj_stk", [P, F_DIM], dt.bfloat16, kind="ExternalInput")
    d_cjl = nc.dram_tensor("cj_lo", [F_DIM, F_DIM], dt.bfloat16, kind="ExternalInput")
    d_jw = nc.dram_tensor("idx_j32", [P, B * T_TILES], dt.int32, kind="ExternalInput")
    d_rel = nc.dram_tensor("idx_rel8", [P, B * T_TILES], dt.int8, kind="ExternalInput")
    d_relT = nc.dram_tensor("idxT_rel8", [P, B * T_TILES * P], dt.int8,
                            kind="ExternalInput")
    d_dif = nc.dram_tensor("diff_w", [P, B * T_TILES, 3], dt.float32,
                           kind="ExternalInput")

    # ---- outputs / scratch ----
    d_ixs = nc.dram_tensor("ixs", [B * CAP, FH], dt.float32, kind="ExternalOutput")
    d_pxo = nc.dram_tensor("px_out_blk", [B, P, FH], dt.float32,
                           kind="ExternalOutput")
    d_pxB = nc.dram_tensor("pxB", [NPXB, FH], dt.float32, kind="Internal")

    with tile.TileContext(nc) as tc:
        with (
            tc.tile_pool(name="const", bufs=1) as cpool,
            tc.tile_pool(name="panel", bufs=1) as panpool,
        ):
            # constants
            ident = cpool.tile([F_DIM, F_DIM], dt.float32, tag="id")
            make_identity(nc, ident[:])
            iota_f8 = cpool.tile([P, P], dt.int8, tag="iof")
            nc.gpsimd.iota(iota_f8[:], pattern=[[1, P]], base=0,
                           channel_multiplier=0,
                           allow_small_or_imprecise_dtypes=True)
            iota_p8 = cpool.tile([P, P], dt.int8, tag="iop")
            nc.gpsimd.iota(iota_p8[:], pattern=[[0, P]], base=0,
                           channel_multiplier=1,
                           allow_small_or_imprecise_dtypes=True)
            t_cis = cpool.tile([P, F_DIM], dt.bfloat16, tag="cis")
            t_cil = cpool.tile([F_DIM, F_DIM], dt.bfloat16, tag="cil")
            t_cjs = cpool.tile([P, F_DIM], dt.bfloat16, tag="cjs")
            t_cjl = cpool.tile([F_DIM, F_DIM], dt.bfloat16, tag="cjl")
            nc.sync.dma_start(t_cis[:], d_cis[:, :])
            nc.sync.dma_start(t_cil[:], d_cil[:, :])
            nc.sync.dma_start(t_cjs[:], d_cjs[:, :])
            nc.sync.dma_start(t_cjl[:], d_cjl[:, :])

            panel_hi = panpool.tile([P, PAN, FH], dt.bfloat16, tag="ph")
            panel_lo = panpool.tile([P, PAN, FH], dt.bfloat16, tag="pl")

            # ---------------- phase 1a: pxA panel ----------------
            with (
                tc.tile_pool(name="p1a", bufs=2) as p1,
                tc.tile_pool(name="p1aps", bufs=2, space="PSUM") as p1ps,
            ):
                for t in range(PB):
                    xt = p1.tile([P, 3, P1_TILE], dt.bfloat16, tag="xt")
                    nc.sync.dma_start(
                        xt[:], d_pxTa[:, :, t * P1_TILE:(t + 1) * P1_TILE])
                    sb_f = p1.tile([F_DIM, 3, P1_TILE], dt.float32, tag="sbf")
                    for v in range(3):
                        ps = p1ps.tile([F_DIM, P1_TILE], dt.float32,
                                       space="PSUM", tag="p1p")
                        nc.tensor.matmul(ps[:, :], lhsT=t_cis[:],
                                         rhs=xt[:, v, :], start=True, stop=False)
                        nc.tensor.matmul(ps[:, :], lhsT=t_cil[:],
                                         rhs=xt[0:F_DIM, v, :], start=False,
                                         stop=True)
                        nc.scalar.copy(sb_f[:, v, :], ps[:])
                    for c in range(4):
                        blk = t * 4 + c
                        for v in range(3):
                            pst = p1ps.tile([P, F_DIM], dt.float32,
                                            space="PSUM", tag="p1t")
                            nc.tensor.transpose(
                                out=pst[:],
                                in_=sb_f[:, v, c * P:(c + 1) * P],
                                identity=ident[:])
                            dst_h = panel_hi[:, blk, v * F_DIM:(v + 1) * F_DIM]
                            dst_l = panel_lo[:, blk, v * F_DIM:(v + 1) * F_DIM]
                            nc.vector.tensor_copy(dst_h, pst[:])
                            nc.vector.tensor_tensor(
                                out=dst_l, in0=pst[:], in1=dst_h,
                                op=mybir.AluOpType.subtract)

            # ---------------- phase 1b: pxB -> DRAM ----------------
            with (
                tc.tile_pool(name="p1b", bufs=3) as p2,
                tc.tile_pool(name="p1bps", bufs=2, space="PSUM") as p2ps,
            ):
                for t in range(NT1B):
                    xt = p2.tile([P, 3, P1_TILE], dt.bfloat16, tag="xt")
                    nc.sync.dma_start(
                        xt[:], d_pxT[:, :, t * P1_TILE:(t + 1) * P1_TILE])
                    sb_f = p2.tile([F_DIM, 3, P1_TILE], dt.float32, tag="sbf")
                    for v in range(3):
                        ps = p2ps.tile([F_DIM, P1_TILE], dt.float32,
                                       space="PSUM", tag="p2p")
                        nc.tensor.matmul(ps[:, :], lhsT=t_cjs[:],
                                         rhs=xt[:, v, :], start=True, stop=False)
                        nc.tensor.matmul(ps[:, :], lhsT=t_cjl[:],
                                         rhs=xt[0:F_DIM, v, :], start=False,
                                         stop=True)
                        nc.scalar.copy(sb_f[:, v, :], ps[:])
                    stg = p2.tile([P, 4, FH], dt.float32, tag="stg")
                    for c in range(4):
                        for v in range(3):
                            pst = p2ps.tile([P, F_DIM], dt.float32,
                                            space="PSUM", tag="p2t")
                            nc.tensor.transpose(
                                out=pst[:],
                                in_=sb_f[:, v, c * P:(c + 1) * P],
                                identity=ident[:])
                            if v % 2 == 0:
                                nc.vector.tensor_copy(
                                    stg[:, c, v * F_DIM:(v + 1) * F_DIM], pst[:])
                            else:
                                nc.scalar.copy(
                                    stg[:, c, v * F_DIM:(v + 1) * F_DIM], pst[:])
                    nc.sync.dma_start(
                        d_pxB[t * P1_TILE:(t + 1) * P1_TILE, :]
                        .rearrange("(s p) f -> p s f", p=P),
                        stg[:, :, :])

            # ---------------- phase 2: edge loop ----------------
            with (
                tc.tile_pool(name="blk", bufs=2) as bp,
                tc.tile_pool(name="oh", bufs=4) as ohp,
                tc.tile_pool(name="psg", bufs=2, space="PSUM") as psgp,
                tc.tile_pool(name="psa", bufs=2, space="PSUM") as psap,
            ):
                for w in range(B):
                    t_jw = bp.tile([P, T_TILES], dt.int32, tag="jw")
                    t_rel = bp.tile([P, T_TILES], dt.int8, tag="rel")
                    t_relT = bp.tile([P, T_TILES * P], dt.int8, tag="relT")
                    t_dif = bp.tile([P, T_TILES, 3], dt.float32, tag="dif")
                    nc.sync.dma_start(t_jw[:], d_jw[:, w * T_TILES:(w + 1) * T_TILES])
                    nc.sync.dma_start(t_rel[:], d_rel[:, w * T_TILES:(w + 1) * T_TILES])
                    nc.sync.dma_start(
                        t_relT[:], d_relT[:, w * T_TILES * P:(w + 1) * T_TILES * P])
                    nc.sync.dma_start(
                        t_dif[:], d_dif[:, w * T_TILES:(w + 1) * T_TILES, :])
                    gB = bp.tile([P, T_TILES, FH], dt.float32, tag="gB")
                    stag = bp.tile([P, T_TILES, FH], dt.float32, tag="stag")
                    ps_acc = psap.tile([P, FH], dt.float32, space="PSUM", tag="acc")

                    for t in range(T_TILES):
                        nc.gpsimd.indirect_dma_start(
                            out=gB[:, t, :], out_offset=None,
                            in_=d_pxB[:, :],
                            in_offset=bass.IndirectOffsetOnAxis(
                                ap=t_jw[:, t:t + 1], axis=0))
                        ohT = ohp.tile([P, P], dt.bfloat16, tag="ohT")
                        nc.vector.tensor_tensor(
                            out=ohT[:], in0=iota_p8[:],
                            in1=t_relT[:, t * P:(t + 1) * P],
                            op=mybir.AluOpType.is_equal)
                        psg = psgp.tile([P, FH], dt.float32, space="PSUM",
                                        tag="psg")
                        nc.tensor.matmul(psg[:], lhsT=ohT[:],
                                         rhs=panel_hi[:, w, :],
                                         start=True, stop=False)
                        nc.tensor.matmul(psg[:], lhsT=ohT[:],
                                         rhs=panel_lo[:, w, :],
                                         start=False, stop=True)
                        nc.vector.tensor_tensor(
                            out=stag[:, t, :], in0=psg[:], in1=gB[:, t, :],
                            op=mybir.AluOpType.add)
                        nc.vector.tensor_tensor(
                            out=stag[:, t, :], in0=stag[:, t, :],
                            in1=t_dif[:, t, :].to_broadcast([P, 3, F_DIM]),
                            op=mybir.AluOpType.mult)
                        ohE = ohp.tile([P, P], dt.float32, tag="ohE")
                        nc.vector.tensor_tensor(
                            out=ohE[:], in0=iota_f8[:],
                            in1=t_rel[:, t:t + 1].to_broadcast([P, P]),
                            op=mybir.AluOpType.is_equal)
                        nc.tensor.matmul(ps_acc[:], lhsT=ohE[:],
                                         rhs=stag[:, t, :],
                                         start=(t == 0), stop=(t == T_TILES - 1))

                    flush = bp.tile([P, FH], dt.float32, tag="flush")
                    nc.scalar.copy(flush[:], ps_acc[:])
                    nc.sync.dma_start(d_pxo[w, :, :], flush[:])
                    nc.sync.dma_start(
                        d_ixs[w * CAP:(w + 1) * CAP, :]
                        .rearrange("(s p) f -> p s f", p=P),
                        stag[:, :, :])
                    nc.gpsimd.dma_reset()

    nc.compile()
    return nc


def kernel(idx_i, idx_j, px, diff, W_pp, W_i, W_j, W_ii):
    idx_i = np.asarray(idx_i).astype(np.int64)
    idx_j = np.asarray(idx_j).astype(np.int64)
    px = np.asarray(px, dtype=np.float32)
    diff = np.asarray(diff, dtype=np.float32)
    E = idx_i.shape[0]
    N = px.shape[0]
    pxf = np.ascontiguousarray(px.reshape(N, FH))
    dif3 = np.ascontiguousarray(diff.reshape(E, 3))

    # folded weights
    C_i = (np.asarray(W_ii, np.float64) @ np.asarray(W_i, np.float64)
           @ np.asarray(W_pp, np.float64)).astype(np.float32)
    C_j = (np.asarray(W_ii, np.float64) @ np.asarray(W_j, np.float64)
           @ np.asarray(W_pp, np.float64)).astype(np.float32)
    cis, cil = _weight_consts(C_i)
    cjs, cjl = _weight_consts(C_j)

    # sort and shard
    order = np.argsort(idx_i, kind="stable")
    si_all = idx_i[order]
    sj_all = idx_j[order]
    sd_all = dif3[order]
    Epc = (E + NC - 1) // NC

    shards = []
    for c in range(NC):
        lo, hi = c * Epc, min((c + 1) * Epc, E)
        shards.append((lo, hi))

    blocks_per_core = []
    for lo, hi in shards:
        blocks_per_core.append(_build_blocks(si_all[lo:hi]))
    B = max(len(b) for b in blocks_per_core)

    # phase-1b geometry (same for all cores)
    NT1B = (N + P1_TILE - 1) // P1_TILE
    NPXB = NT1B * P1_TILE
    pxT_stk = _stack_pxT(pxf, cols_idx=None, ncols=NPXB)

    in_maps = []
    core_meta = []
    for c, (lo, hi) in enumerate(shards):
        si = si_all[lo:hi]
        sj = sj_all[lo:hi]
        sd = sd_all[lo:hi]
        op = order[lo:hi]
        blocks = blocks_per_core[c]

        L = B * CAP
        jw = np.zeros(L, np.int32)
        rel = np.zeros(L, np.int8)
        dfw = np.zeros((L, 3), np.float32)
        perm = np.full(L, -1, np.int64)
        bases = np.full(B, -1, np.int64)
        pan_cols = np.full(B * P, -1, np.int64)   # node id per panel slot
        for b, (n0, s, e) in enumerate(blocks):
            n_e = e - s
            o = b * CAP
            jw[o:o + n_e] = sj[s:e]
            rel[o:o + n_e] = (si[s:e] - n0).astype(np.int8)
            dfw[o:o + n_e] = sd[s:e]
            perm[o:o + n_e] = op[s:e]
            bases[b] = n0
            pan_cols[b * P:(b + 1) * P] = np.arange(n0, n0 + P)

        PAN = ((B + 3) // 4) * 4
        pc = np.full(PAN * P, -1, np.int64)
        pc[:B * P] = pan_cols
        pxTa_stk = _stack_pxT(pxf, cols_idx=pc, ncols=PAN * P)

        relT = np.broadcast_to(rel[None, :], (P, L))

        in_maps.append(dict(
            pxT_stk=pxT_stk,
            pxTa_stk=pxTa_stk,
            ci_stk=cis, ci_lo=cil, cj_stk=cjs, cj_lo=cjl,
            idx_j32=_wrap_tiles(jw).reshape(P, B * T_TILES),
            idx_rel8=_wrap_tiles(rel).reshape(P, B * T_TILES),
            idxT_rel8=np.ascontiguousarray(relT),
            diff_w=_wrap_tiles(dfw, inner=3).reshape(P, B * T_TILES, 3),
        ))
        core_meta.append(dict(perm=perm, bases=bases, nblocks=len(blocks)))

    nc = _build_bass(B, NT1B, NPXB)

    from concourse.bass_utils import run_bass_kernel_spmd
    import os
    trace = bool(os.environ.get("KERNEL_TRACE"))
    res = run_bass_kernel_spmd(nc, in_maps, core_ids=list(range(NC)),
                               trace=trace)
    if trace:
        global LAST_EXEC_NS
        LAST_EXEC_NS = res.exec_time_ns

    # ---- host assembly ----
    px_out = np.zeros((N + P, FH), np.float32)
    ix = np.empty((E, FH), np.float32)
    for c in range(NC):
        r = res.results[c]
        meta = core_meta[c]
        perm = meta["perm"]
        mask = perm >= 0
        ix[perm[mask]] = r["ixs"][mask]
        blk = r["px_out_blk"]
        for b in range(meta["nblocks"]):
            n0 = meta["bases"][b]
            px_out[n0:n0 + P] += blk[b]
    px_out = px_out[:N]
    return (px_out.reshape(N, 3, F_DIM),
            ix.reshape(E, 3, F_DIM))


LAST_EXEC_NS = None


# revision 5
# speedup vs baseline: 1.0954x; 1.0954x over previous
"""Trainium2 Bass kernel for nn_EqvarLayer (GNN message passing).

Computes, for the reference module:
    px1    = px @ W_pp.T                      (node-level linear)
    ix     = (px1[idx_i] @ W_i.T + px1[idx_j] @ W_j.T) * diff @ W_ii.T
    px_out = segment_sum(ix, idx_i)

All linear maps commute with the gather and the per-(edge,v) diff scale, so
with C_i = W_ii@W_i@W_pp and C_j = W_ii@W_j@W_pp (folded on host):
    ix[e,v,:]  = diff[e,v] * (pxA[idx_i[e],v,:] + pxB[idx_j[e],v,:])
    pxA = px @ C_i.T ; pxB = px @ C_j.T       (node-level, computed on device)

Edge-parallel across 8 NeuronCores: edges are sorted by idx_i on the host and
sharded contiguously.  Within a core the sorted edges are grouped into node
"blocks" (<=128-node span, fixed tile count T).  Per 128-edge tile:
  - idx_j side: one [128,1]-indexed indirect DMA gathers pxB rows (768B each)
  - idx_i side: a one-hot matmul against an SBUF-resident pxA panel (exact:
    pxA is kept as a bf16 hi+lo pair; two accumulating matmuls reconstruct
    the f32 value in PSUM)
  - stag = (pxA_g + pxB_g) * diff  on DVE (broadcast AP for the diff expand)
  - segment-sum: one-hot^T matmul accumulated in PSUM across the block's
    tiles, flushed once per block (no DRAM read-modify-write at all)
The scatter output is returned block-major and assembled on the host.
"""

import numpy as np
import ml_dtypes

N_NODES = 50000
N_EDGES = 800000
F_DIM = 64
FH = 3 * F_DIM            # 192 floats per node/edge row
NC = 8                    # cores
P = 128
T_TILES = 14              # tiles (of 128 edges) per block
CAP = T_TILES * P         # max edges per block
P1_TILE = 512             # phase-1 node tile (columns per matmul)

BF16 = ml_dtypes.bfloat16


def _split_hilo(x32):
    hi = x32.astype(BF16)
    lo = (x32 - hi.astype(np.float32)).astype(BF16)
    return hi, lo


def _build_blocks(si):
    """Partition sorted node ids si into blocks of <=CAP edges spanning
    <128 node ids.  Returns list of (n0, start, end)."""
    blocks = []
    s = 0
    n = len(si)
    while s < n:
        n0 = int(si[s])
        hi = int(np.searchsorted(si, n0 + P, side="left"))
        e = min(s + CAP, hi)
        blocks.append((n0, s, e))
        s = e
    return blocks


def _wrap_tiles(arr, inner=None):
    """[L(,inner)] -> [128, L//128(,inner)], edge k*128+p at [p,k]."""
    if inner is None:
        return np.ascontiguousarray(arr.reshape(-1, P).T)
    return np.ascontiguousarray(arr.reshape(-1, P, inner).transpose(1, 0, 2))


def _stack_pxT(pxf, cols_idx=None, ncols=None):
    """Build the [128, 3, ncols] bf16 stacked (hi/lo interleaved) feature-major
    layout from px rows.  pxf: [N, 192] f32.  Row r of v-block: r<64 -> hi of
    feature r, r>=64 -> lo of feature r-64."""
    if cols_idx is None:
        sel = pxf
    else:
        sel = np.zeros((len(cols_idx), FH), np.float32)
        valid = (cols_idx >= 0) & (cols_idx < pxf.shape[0])
        sel[valid] = pxf[cols_idx[valid]]
    n = sel.shape[0]
    if ncols is None:
        ncols = n
    xT = sel.T.reshape(3, F_DIM, n)                  # [v, f, n]
    hi = xT.astype(BF16)
    lo = (xT - hi.astype(np.float32)).astype(BF16)
    out = np.zeros((P, 3, ncols), BF16)
    out[:F_DIM, :, :n] = hi.transpose(1, 0, 2)       # [f, v, n]
    out[F_DIM:, :, :n] = lo.transpose(1, 0, 2)
    return out


def _weight_consts(C):
    """lhsT constants for the hi/lo matmul pair of out = C @ x."""
    Chi = C.astype(BF16)
    Clo = (C - Chi.astype(np.float32)).astype(BF16)
    stk = np.zeros((P, F_DIM), BF16)
    stk[:F_DIM] = Chi.T                               # [f, g]
    stk[F_DIM:] = Chi.T
    return np.ascontiguousarray(stk), np.ascontiguousarray(Clo.T)


def _build_bass(B, NT1B, NPXB):
    """Build the SPMD bass program.  B: blocks per core; NT1B: phase-1b node
    tiles; NPXB: padded pxB row count."""
    import concourse.bass as bass
    import concourse.bacc as bacc
    import concourse.mybir as mybir
    import concourse.tile as tile
    from concourse.masks import make_identity

    dt = mybir.dt
    PB = (B + 3) // 4                 # phase-1a 512-node tiles
    PAN = PB * 4                      # panel block slots (padded to x4)

    nc = bacc.Bacc(None, target_bir_lowering=False,
                   dynamic_dma_scratch_size=32768)

    # ---- inputs ----
    d_pxT = nc.dram_tensor("pxT_stk", [P, 3, NT1B * P1_TILE], dt.bfloat16,
                           kind="ExternalInput")
    d_pxTa = nc.dram_tensor("pxTa_stk", [P, 3, PAN * P], dt.bfloat16,
                            kind="ExternalInput")
    d_cis = nc.dram_tensor("ci_stk", [P, F_DIM], dt.bfloat16, kind="ExternalInput")
    d_cil = nc.dram_tensor("ci_lo", [F_DIM, F_DIM], dt.bfloat16, kind="ExternalInput")
    d_cjs = nc.dram_tensor("cj_stk", [P, F_DIM], dt.bfloat16, kind="ExternalInput")
    d_cjl = nc.dram_tensor("cj_lo", [F_DIM, F_DIM], dt.bfloat16, kind="ExternalInput")
    d_jw = nc.dram_tensor("idx_j32", [P, B * T_TILES], dt.int32, kind="ExternalInput")
    d_rel = nc.dram_tensor("idx_rel16", [P, B * T_TILES], dt.int16, kind="ExternalInput")
    d_relT = nc.dram_tensor("idxT_rel16", [P, B * T_TILES * P], dt.int16,
                            kind="ExternalInput")
    d_dif = nc.dram_tensor("diff_w", [P, B * T_TILES, 3], dt.float32,
                           kind="ExternalInput")

    # ---- outputs / scratch ----
    d_ixs = nc.dram_tensor("ixs", [B * CAP, FH], dt.float32, kind="ExternalOutput")
    d_pxo = nc.dram_tensor("px_out_blk", [B, P, FH], dt.float32,
                           kind="ExternalOutput")
    d_pxB = nc.dram_tensor("pxB", [NPXB, FH], dt.float32, kind="Internal")

    with tile.TileContext(nc) as tc:
        with (
            tc.tile_pool(name="const", bufs=1) as cpool,
            tc.tile_pool(name="panel", bufs=1) as panpool,
        ):
            # constants
            ident = cpool.tile([P, P], dt.float32, tag="id")
            make_identity(nc, ident[:])
            iota_f8 = cpool.tile([P, P], dt.int16, tag="iof")
            nc.gpsimd.iota(iota_f8[:], pattern=[[1, P]], base=0,
                           channel_multiplier=0)
            iota_p8 = cpool.tile([P, P], dt.int16, tag="iop")
            nc.gpsimd.iota(iota_p8[:], pattern=[[0, P]], base=0,
                           channel_multiplier=1)
            t_cis = cpool.tile([P, F_DIM], dt.bfloat16, tag="cis")
            t_cil = cpool.tile([F_DIM, F_DIM], dt.bfloat16, tag="cil")
            t_cjs = cpool.tile([P, F_DIM], dt.bfloat16, tag="cjs")
            t_cjl = cpool.tile([F_DIM, F_DIM], dt.bfloat16, tag="cjl")
            nc.sync.dma_start(t_cis[:], d_cis[:, :])
            nc.sync.dma_start(t_cil[:], d_cil[:, :])
            nc.sync.dma_start(t_cjs[:], d_cjs[:, :])
            nc.sync.dma_start(t_cjl[:], d_cjl[:, :])

            panel_hi = panpool.tile([P, PAN, FH], dt.bfloat16, tag="ph")
            panel_lo = panpool.tile([P, PAN, FH], dt.bfloat16, tag="pl")

            # ---------------- phase 1a: pxA panel ----------------
            with (
                tc.tile_pool(name="p1a", bufs=2) as p1,
                tc.tile_pool(name="p1aps", bufs=2, space="PSUM") as p1ps,
            ):
                for t in range(PB):
                    xt = p1.tile([P, 3, P1_TILE], dt.bfloat16, tag="xt")
                    nc.sync.dma_start(
                        xt[:], d_pxTa[:, :, t * P1_TILE:(t + 1) * P1_TILE])
                    sb01 = p1.tile([P, P1_TILE], dt.float32, tag="sb01")
                    sb2 = p1.tile([F_DIM, P1_TILE], dt.float32, tag="sb2")
                    ps01 = p1ps.tile([P, P1_TILE], dt.float32,
                                     space="PSUM", tag="p1p")
                    for v in range(2):
                        nc.tensor.matmul(ps01[v * F_DIM:(v + 1) * F_DIM, :],
                                         lhsT=t_cis[:], rhs=xt[:, v, :],
                                         start=True, stop=False,
                                         tile_position=(0, v * F_DIM))
                        nc.tensor.matmul(ps01[v * F_DIM:(v + 1) * F_DIM, :],
                                         lhsT=t_cil[:], rhs=xt[0:F_DIM, v, :],
                                         start=False, stop=True,
                                         tile_position=(0, v * F_DIM))
                    nc.scalar.copy(sb01[:], ps01[:])
                    ps2 = p1ps.tile([F_DIM, P1_TILE], dt.float32,
                                    space="PSUM", tag="p1q")
                    nc.tensor.matmul(ps2[:, :], lhsT=t_cis[:],
                                     rhs=xt[:, 2, :], start=True, stop=False)
                    nc.tensor.matmul(ps2[:, :], lhsT=t_cil[:],
                                     rhs=xt[0:F_DIM, 2, :], start=False,
                                     stop=True)
                    nc.scalar.copy(sb2[:], ps2[:])
                    for c in range(4):
                        blk = t * 4 + c
                        pst = p1ps.tile([P, P], dt.float32,
                                        space="PSUM", tag="p1t")
                        nc.tensor.transpose(
                            out=pst[:], in_=sb01[:, c * P:(c + 1) * P],
                            identity=ident[:])
                        pst2 = p1ps.tile([P, F_DIM], dt.float32,
                                         space="PSUM", tag="p1u")
                        nc.tensor.transpose(
                            out=pst2[:], in_=sb2[:, c * P:(c + 1) * P],
                            identity=ident[0:F_DIM, 0:F_DIM])
                        dst_h = panel_hi[:, blk, 0:P]
                        dst_l = panel_lo[:, blk, 0:P]
                        nc.vector.tensor_copy(dst_h, pst[:])
                        nc.vector.tensor_tensor(
                            out=dst_l, in0=pst[:], in1=dst_h,
                            op=mybir.AluOpType.subtract)
                        dst_h2 = panel_hi[:, blk, P:P + F_DIM]
                        dst_l2 = panel_lo[:, blk, P:P + F_DIM]
                        nc.vector.tensor_copy(dst_h2, pst2[:])
                        nc.vector.tensor_tensor(
                            out=dst_l2, in0=pst2[:], in1=dst_h2,
                            op=mybir.AluOpType.subtract)

            # ---------------- phase 1b: pxB -> DRAM ----------------
            with (
                tc.tile_pool(name="p1b", bufs=3) as p2,
                tc.tile_pool(name="p1bps", bufs=2, space="PSUM") as p2ps,
            ):
                for t in range(NT1B):
                    xt = p2.tile([P, 3, P1_TILE], dt.bfloat16, tag="xt")
                    nc.sync.dma_start(
                        xt[:], d_pxT[:, :, t * P1_TILE:(t + 1) * P1_TILE])
                    sb01 = p2.tile([P, P1_TILE], dt.float32, tag="sb01")
                    sb2 = p2.tile([F_DIM, P1_TILE], dt.float32, tag="sb2")
                    ps01 = p2ps.tile([P, P1_TILE], dt.float32,
                                     space="PSUM", tag="p2p")
                    for v in range(2):
                        nc.tensor.matmul(ps01[v * F_DIM:(v + 1) * F_DIM, :],
                                         lhsT=t_cjs[:], rhs=xt[:, v, :],
                                         start=True, stop=False,
                                         tile_position=(0, v * F_DIM))
                        nc.tensor.matmul(ps01[v * F_DIM:(v + 1) * F_DIM, :],
                                         lhsT=t_cjl[:], rhs=xt[0:F_DIM, v, :],
                                         start=False, stop=True,
                                         tile_position=(0, v * F_DIM))
                    nc.scalar.copy(sb01[:], ps01[:])
                    ps2 = p2ps.tile([F_DIM, P1_TILE], dt.float32,
                                    space="PSUM", tag="p2q")
                    nc.tensor.matmul(ps2[:, :], lhsT=t_cjs[:],
                                     rhs=xt[:, 2, :], start=True, stop=False)
                    nc.tensor.matmul(ps2[:, :], lhsT=t_cjl[:],
                                     rhs=xt[0:F_DIM, 2, :], start=False,
                                     stop=True)
                    nc.scalar.copy(sb2[:], ps2[:])
                    stg = p2.tile([P, 4, FH], dt.float32, tag="stg")
                    for c in range(4):
                        pst = p2ps.tile([P, P], dt.float32,
                                        space="PSUM", tag="p2t")
                        nc.tensor.transpose(
                            out=pst[:], in_=sb01[:, c * P:(c + 1) * P],
                            identity=ident[:])
                        if c % 2 == 0:
                            nc.vector.tensor_copy(stg[:, c, 0:P], pst[:])
                        else:
                            nc.scalar.copy(stg[:, c, 0:P], pst[:])
                        pst2 = p2ps.tile([P, F_DIM], dt.float32,
                                         space="PSUM", tag="p2u")
                        nc.tensor.transpose(
                            out=pst2[:], in_=sb2[:, c * P:(c + 1) * P],
                            identity=ident[0:F_DIM, 0:F_DIM])
                        if c % 2 == 0:
                            nc.scalar.copy(stg[:, c, P:P + F_DIM], pst2[:])
                        else:
                            nc.vector.tensor_copy(stg[:, c, P:P + F_DIM], pst2[:])
                    nc.sync.dma_start(
                        d_pxB[t * P1_TILE:(t + 1) * P1_TILE, :]
                        .rearrange("(s p) f -> p s f", p=P),
                        stg[:, :, :])

            # ---------------- phase 2: edge loop ----------------
            with (
                tc.tile_pool(name="blk", bufs=2) as bp,
                tc.tile_pool(name="oh", bufs=4) as ohp,
                tc.tile_pool(name="psg", bufs=2, space="PSUM") as psgp,
                tc.tile_pool(name="psa", bufs=2, space="PSUM") as psap,
            ):
                for w in range(B):
                    t_jw = bp.tile([P, T_TILES], dt.int32, tag="jw")
                    t_rel = bp.tile([P, T_TILES], dt.int16, tag="rel")
                    t_relT = bp.tile([P, T_TILES * P], dt.int16, tag="relT")
                    t_dif = bp.tile([P, T_TILES, 3], dt.float32, tag="dif")
                    nc.sync.dma_start(t_jw[:], d_jw[:, w * T_TILES:(w + 1) * T_TILES])
                    nc.sync.dma_start(t_rel[:], d_rel[:, w * T_TILES:(w + 1) * T_TILES])
                    nc.sync.dma_start(
                        t_relT[:], d_relT[:, w * T_TILES * P:(w + 1) * T_TILES * P])
                    nc.sync.dma_start(
                        t_dif[:], d_dif[:, w * T_TILES:(w + 1) * T_TILES, :])
                    gB = bp.tile([P, T_TILES, FH], dt.float32, tag="gB")
                    stag = bp.tile([P, T_TILES, FH], dt.float32, tag="stag")
                    ps_acc = psap.tile([P, FH], dt.float32, space="PSUM", tag="acc")

                    for t in range(T_TILES):
                        nc.gpsimd.indirect_dma_start(
                            out=gB[:, t, :], out_offset=None,
                            in_=d_pxB[:, :],
                            in_offset=bass.IndirectOffsetOnAxis(
                                ap=t_jw[:, t:t + 1], axis=0))
                        ohT = ohp.tile([P, P], dt.bfloat16, tag="ohT")
                        nc.vector.tensor_tensor(
                            out=ohT[:], in0=iota_p8[:],
                            in1=t_relT[:, t * P:(t + 1) * P],
                            op=mybir.AluOpType.is_equal)
                        psg = psgp.tile([P, FH], dt.float32, space="PSUM",
                                        tag="psg")
                        nc.tensor.matmul(psg[:], lhsT=ohT[:],
                                         rhs=panel_hi[:, w, :],
                                         start=True, stop=False)
                        nc.tensor.matmul(psg[:], lhsT=ohT[:],
                                         rhs=panel_lo[:, w, :],
                                         start=False, stop=True)
                        nc.vector.tensor_tensor(
                            out=stag[:, t, :], in0=psg[:], in1=gB[:, t, :],
                            op=mybir.AluOpType.add)
                        for v in range(3):
                            nc.scalar.activation(
                                stag[:, t, v * F_DIM:(v + 1) * F_DIM],
                                stag[:, t, v * F_DIM:(v + 1) * F_DIM],
                                mybir.ActivationFunctionType.Copy,
                                scale=t_dif[:, t, v:v + 1])
                        ohE = ohp.tile([P, P], dt.float32, tag="ohE")
                        nc.vector.tensor_tensor(
                            out=ohE[:], in0=iota_f8[:],
                            in1=t_rel[:, t:t + 1].to_broadcast([P, P]),
                            op=mybir.AluOpType.is_equal)
                        nc.tensor.matmul(ps_acc[:], lhsT=ohE[:],
                                         rhs=stag[:, t, :],
                                         start=(t == 0), stop=(t == T_TILES - 1))

                    flush = bp.tile([P, FH], dt.float32, tag="flush")
                    nc.scalar.copy(flush[:], ps_acc[:])
                    nc.sync.dma_start(d_pxo[w, :, :], flush[:])
                    nc.sync.dma_start(
                        d_ixs[w * CAP:(w + 1) * CAP, :]
                        .rearrange("(s p) f -> p s f", p=P),
                        stag[:, :, :])
                    nc.gpsimd.dma_reset()

    nc.compile()
    return nc


def kernel(idx_i, idx_j, px, diff, W_pp, W_i, W_j, W_ii):
    idx_i = np.asarray(idx_i).astype(np.int64)
    idx_j = np.asarray(idx_j).astype(np.int64)
    px = np.asarray(px, dtype=np.float32)
    diff = np.asarray(diff, dtype=np.float32)
    E = idx_i.shape[0]
    N = px.shape[0]
    pxf = np.ascontiguousarray(px.reshape(N, FH))
    dif3 = np.ascontiguousarray(diff.reshape(E, 3))

    # folded weights
    C_i = (np.asarray(W_ii, np.float64) @ np.asarray(W_i, np.float64)
           @ np.asarray(W_pp, np.float64)).astype(np.float32)
    C_j = (np.asarray(W_ii, np.float64) @ np.asarray(W_j, np.float64)
           @ np.asarray(W_pp, np.float64)).astype(np.float32)
    cis, cil = _weight_consts(C_i)
    cjs, cjl = _weight_consts(C_j)

    # sort and shard
    order = np.argsort(idx_i, kind="stable")
    si_all = idx_i[order]
    sj_all = idx_j[order]
    sd_all = dif3[order]
    Epc = (E + NC - 1) // NC

    shards = []
    for c in range(NC):
        lo, hi = c * Epc, min((c + 1) * Epc, E)
        shards.append((lo, hi))

    blocks_per_core = []
    for lo, hi in shards:
        blocks_per_core.append(_build_blocks(si_all[lo:hi]))
    B = max(len(b) for b in blocks_per_core)

    # phase-1b geometry (same for all cores)
    NT1B = (N + P1_TILE - 1) // P1_TILE
    NPXB = NT1B * P1_TILE
    pxT_stk = _stack_pxT(pxf, cols_idx=None, ncols=NPXB)

    in_maps = []
    core_meta = []
    for c, (lo, hi) in enumerate(shards):
        si = si_all[lo:hi]
        sj = sj_all[lo:hi]
        sd = sd_all[lo:hi]
        op = order[lo:hi]
        blocks = blocks_per_core[c]

        L = B * CAP
        jw = np.zeros(L, np.int32)
        rel = np.zeros(L, np.int16)
        dfw = np.zeros((L, 3), np.float32)
        perm = np.full(L, -1, np.int64)
        bases = np.full(B, -1, np.int64)
        pan_cols = np.full(B * P, -1, np.int64)   # node id per panel slot
        for b, (n0, s, e) in enumerate(blocks):
            n_e = e - s
            o = b * CAP
            jw[o:o + n_e] = sj[s:e]
            rel[o:o + n_e] = (si[s:e] - n0).astype(np.int16)
            dfw[o:o + n_e] = sd[s:e]
            perm[o:o + n_e] = op[s:e]
            bases[b] = n0
            pan_cols[b * P:(b + 1) * P] = np.arange(n0, n0 + P)

        PAN = ((B + 3) // 4) * 4
        pc = np.full(PAN * P, -1, np.int64)
        pc[:B * P] = pan_cols
        pxTa_stk = _stack_pxT(pxf, cols_idx=pc, ncols=PAN * P)

        relT = np.broadcast_to(rel[None, :], (P, L))

        in_maps.append(dict(
            pxT_stk=pxT_stk,
            pxTa_stk=pxTa_stk,
            ci_stk=cis, ci_lo=cil, cj_stk=cjs, cj_lo=cjl,
            idx_j32=_wrap_tiles(jw).reshape(P, B * T_TILES),
            idx_rel16=_wrap_tiles(rel).reshape(P, B * T_TILES),
            idxT_rel16=np.ascontiguousarray(relT),
            diff_w=_wrap_tiles(dfw, inner=3).reshape(P, B * T_TILES, 3),
        ))
        core_meta.append(dict(perm=perm, bases=bases, nblocks=len(blocks)))

    nc = _build_bass(B, NT1B, NPXB)

    from concourse.bass_utils import run_bass_kernel_spmd
    import os
    trace = bool(os.environ.get("KERNEL_TRACE"))
    res = run_bass_kernel_spmd(nc, in_maps, core_ids=list(range(NC)),
                               trace=trace)
    if trace:
        global LAST_EXEC_NS
        LAST_EXEC_NS = res.exec_time_ns

    # ---- host assembly ----
    px_out = np.zeros((N + P, FH), np.float32)
    ix = np.empty((E, FH), np.float32)
    for c in range(NC):
        r = res.results[c]
        meta = core_meta[c]
        perm = meta["perm"]
        mask = perm >= 0
        ix[perm[mask]] = r["ixs"][mask]
        blk = r["px_out_blk"]
        for b in range(meta["nblocks"]):
            n0 = meta["bases"][b]
            px_out[n0:n0 + P] += blk[b]
    px_out = px_out[:N]
    return (px_out.reshape(N, 3, F_DIM),
            ix.reshape(E, 3, F_DIM))


LAST_EXEC_NS = None
